# revision 19
# baseline (speedup 1.0000x reference)
"""Trainium2 Bass kernel for nn_PitchRegisterTracker.

Algorithm notes
---------------
The reference maintains a size-1000 circular buffer of log2-pitches of the
valid (>0) frames, then normalizes every valid frame by the buffer's
mean/unbiased-std.  Because slot j keeps the *highest-rank* writer, the full
buffer is exactly the last min(1000, n_valid) valid elements.  So:

  phase A: stats (mean/std of ln-pitch) over the last 1000 valid elements.
           Only a small tail window of the input can contain them; every core
           redundantly computes the same stats from the same tail (no
           collectives needed).
  phase B: fully data-parallel elementwise map
           out = exp(sc * ln(p) + bi),  out(0) = 0 via exp(ln(0)) = exp(-inf).

where, with ln-space stats meanL/stdL over the selected values:
  sc = TLS*ln2/stdL   bi = ln2*TLM - meanL*sc
matching the reference's exp2((log2p - mean2)/std2 * TLS + TLM).

Performance notes
-----------------
The kernel is HBM-bandwidth bound (16 MiB in + 16 MiB out per core).  HWDGE
splits one DMA's descriptors over d SDMA engines, d = largest divisor of the
OUTERMOST access-pattern dim <= 16, in contiguous chunks from engine 0; one
descriptor costs ~45ns fixed + bytes/~28.4GB/s on its engine, so descriptors
want to be as large as possible (<64 KiB) and the outer dim a multiple of 16.

Key profiling findings baked in here:
 - Any SWDGE (gpsimd) DMA use makes SDMA engine 15 ~20% slower for the WHOLE
   run (descriptor-ring AXI port contention) - everything then waits on
   engine 15's backlog.  So nothing touches gpsimd: phase-A inputs ride the
   head of SP's HWDGE ring instead.
 - ACT (Ln+Exp = ~54us busy) paces store availability, so tiles are loaded
   smallest-first (ACT starts early) and activated in ~4-5K-column slices,
   each slice released to its own store DMA.
 - All streaming loads+stores ride SP's ring: loads queued first, stores
   strictly after (FIFO keeps the bus work-conserving), ACT does only math.
"""

import sys

for _p in ("/opt/trn_rl_repo", "/root/.axon_site/_ro/trn_rl_repo"):
    if _p not in sys.path:
        sys.path.insert(0, _p)

import math

import numpy as np

import concourse.bass as bass
import concourse.mybir as mybir
from concourse import tile
from concourse.bass_utils import run_bass_kernel_spmd

AF = mybir.ActivationFunctionType
OP = mybir.AluOpType
F32 = mybir.dt.float32

N_CORES = 8
BUF = 1000
LN2 = 0.693147  # the reference's constant, used only inside TLS
TARGET_LOG_MEAN = float(np.log2(200.0))
TARGET_LOG_STD = 40.0 / (200.0 * LN2)
LN2_T = float(np.log(2.0))  # true ln 2

# Loads use the natural [128 x 32768] per-core layout in uniform [128, w]
# tiles - all 16 engines at full rate.  The engine-15 skew lives in the
# STORES only: profiling shows [120]/[8]-outer (non-power-of-2 engine count)
# DMAs run at half rate as loads but full rate as stores.  Stores for the
# first O1C columns go to region O1 as [128, w] (16 engines); the remaining
# O23C columns go to O2 [120 x O23C] + O3 [8 x O23C] (engines 0-14 / 0-7),
# giving engine 15 ~0.74x the bytes of the others - insurance for the runs
# where engine 15 degrades ~25%.  The host reassembles the output from
# O1|O2|O3 with three strided copies.
COLS = 32768
SHARD = 128 * COLS
assert SHARD * N_CORES == 1 << 25

# (width, [slice widths], [slice store targets])
PLAN = [
    (4096, [4096], ["O1"]),
    (8192, [4096, 4096], ["O1", "O1"]),
    (10240, [5120, 5120], ["O1", "O23"]),
    (10240, [5120, 5120], ["O23", "O23"]),
]
O1C = sum(sw for w, ss, tg in PLAN for sw, t in zip(ss, tg) if t == "O1")
O23C = COLS - O1C
assert sum(w for w, ss, tg in PLAN) == COLS


def _legalize_sync_waits(nc, maxw=1):
    """This container's walrus accepts at most one sync-wait command per
    instruction; split extra waits into preceding same-engine NOPs."""
    n = 0
    for f in nc.m.functions:
        for bb in f.blocks:
            insts = bb.instructions
            newlist = []
            for inst in insts:
                si = inst.sync_info
                if si is not None and si.on_wait and len(si.on_wait) > maxw:
                    waits = list(si.on_wait)
                    rest = waits[-maxw:]
                    head = waits[:-maxw]
                    k = 0
                    while head:
                        chunk, head = head[:maxw], head[maxw:]
                        nop = mybir.InstNoOp(
                            name=f"{inst.name}-ws{k}",
                            sync_info=mybir.SyncInfo(
                                on_wait=list(chunk), on_update=[]
                            ),
                            engine=inst.engine,
                            bass_nofuse=True,
                        )
                        nc.register_instruction(nop)
                        newlist.append(nop)
                        k += 1
                        n += 1
                    si.on_wait.clear()
                    si.on_wait.extend(rest)
                newlist.append(inst)
            insts[:] = newlist
    return n


def _build_program(shard, tail):
    """One SPMD program: per-core shard [shard] -> out split into the
    O1/O2/O3 store regions, with the global tail [tail] + constants
    replicated to every core."""
    tailc = tail // 128
    assert shard == SHARD

    nc = bass.Bass()
    xs = nc.dram_tensor("xs", [shard], F32, kind="ExternalInput")
    xt = nc.dram_tensor("xt", [tail], F32, kind="ExternalInput")
    lts = nc.dram_tensor("lts", [128, 256], F32, kind="ExternalInput")
    y1 = nc.dram_tensor("y1", [128 * O1C], F32, kind="ExternalOutput")
    y2 = nc.dram_tensor("y2", [120 * O23C], F32, kind="ExternalOutput")
    y3 = nc.dram_tensor("y3", [8 * O23C], F32, kind="ExternalOutput")

    xst = xs.rearrange("(p c) -> p c", p=128)
    y1t = y1.rearrange("(p c) -> p c", p=128)
    y2t = y2.rearrange("(p c) -> p c", p=120)
    y3t = y3.rearrange("(p c) -> p c", p=8)
    xtt = xt.rearrange("(p c) -> p c", p=128)

    sc_clamp = TARGET_LOG_STD * 1e7
    exp_bias = float(np.log(TARGET_LOG_STD * LN2_T))
    ln2sq = LN2_T * LN2_T
    with tile.TileContext(nc) as tc:
        with (
            tc.tile_pool(name="const", bufs=1) as cpool,
            tc.tile_pool(name="stat", bufs=1) as spool,
            tc.tile_pool(name="psum", bufs=1, space="PSUM") as ppool,
            tc.tile_pool(name="big", bufs=1) as bpool,
        ):
            # ---------------- phase A: stats over last BUF valid in tail
            # phase-A inputs ride the HEAD of SP's ring: they are tiny
            # (~0.2 MB) and the DVE->ACT stats chain is latency-critical
            ltst = cpool.tile([128, 256], F32)
            tailt = cpool.tile([128, tailc], F32)
            nc.sync.dma_start(ltst[:], lts[:])
            nc.sync.dma_start(tailt[:], xtt[:])

            zer = cpool.tile([128, tailc], F32)
            nc.vector.memset(zer[:], 0.0)

            mask = cpool.tile([128, tailc], F32)
            nc.vector.tensor_scalar(mask[:], tailt[:], 0.0, None, OP.is_gt)
            t1 = cpool.tile([128, tailc], F32)
            nc.vector.tensor_scalar(t1[:], tailt[:], 1.0, None, OP.max)
            lnp = cpool.tile([128, tailc], F32)
            nc.scalar.activation(lnp[:], t1[:], AF.Ln)

            # per-partition inclusive prefix count of valid
            pre = cpool.tile([128, tailc], F32)
            nc.vector.tensor_tensor_scan(
                pre[:], mask[:], zer[:], 0.0, OP.add, OP.add
            )

            # cross-partition exclusive prefix + total, via PE matmuls
            ps_rexc = ppool.tile([128, 1], F32)
            ps_vb = ppool.tile([128, 1], F32)
            last = pre[:, tailc - 1 : tailc]
            nc.tensor.matmul(ps_rexc[:], ltst[:, 0:128], last)
            nc.tensor.matmul(ps_vb[:], ltst[:, 128:256], last)
            rexc = spool.tile([128, 1], F32)
            vb = spool.tile([128, 1], F32)
            nc.vector.tensor_copy(rexc[:], ps_rexc[:])
            nc.vector.tensor_copy(vb[:], ps_vb[:])

            # w = V - rexc - BUF ; select valid lanes with global prefix > w
            w = spool.tile([128, 1], F32)
            nc.vector.tensor_scalar(
                w[:], vb[:], rexc[:, 0:1], float(BUF), OP.subtract, OP.subtract
            )
            selg = cpool.tile([128, tailc], F32)
            nc.vector.tensor_scalar(selg[:], pre[:], w[:, 0:1], None, OP.is_gt)
            sel = cpool.tile([128, tailc], F32)
            nc.vector.tensor_tensor(sel[:], selg[:], mask[:], OP.mult)

            # one-pass moments: cnt, sum(t), sum(t^2) over selected lanes,
            # rows packed into one [128,3] tile -> single broadcast matmul
            stats = spool.tile([128, 3], F32)
            slog = cpool.tile([128, tailc], F32)
            slog2 = cpool.tile([128, tailc], F32)
            nc.vector.tensor_reduce(
                stats[:, 0:1], sel[:], mybir.AxisListType.X, OP.add
            )
            nc.vector.tensor_tensor(slog[:], sel[:], lnp[:], OP.mult)
            nc.vector.tensor_reduce(
                stats[:, 1:2], slog[:], mybir.AxisListType.X, OP.add
            )
            nc.vector.tensor_tensor(slog2[:], slog[:], lnp[:], OP.mult)
            nc.vector.tensor_reduce(
                stats[:, 2:3], slog2[:], mybir.AxisListType.X, OP.add
            )
            ps_st = ppool.tile([128, 3], F32)
            nc.tensor.matmul(ps_st[:], ltst[:, 128:256], stats[:, 0:3])
            bst = spool.tile([128, 3], F32)
            nc.vector.tensor_copy(bst[:], ps_st[:])
            cntb = bst[:, 0:1]
            s1b = bst[:, 1:2]
            s2b = bst[:, 2:3]

            # 1/x via exp(-ln x) on ACT: this walrus rejects the custom-DVE
            # reciprocal encoding, and x (a count >= 1) is exact enough here
            cfl = spool.tile([128, 1], F32)
            nc.vector.tensor_scalar(cfl[:], cntb, 1.0, None, OP.max)
            lncf = spool.tile([128, 1], F32)
            nc.scalar.activation(lncf[:], cfl[:], AF.Ln)
            rcp1 = spool.tile([128, 1], F32)
            nc.scalar.activation(rcp1[:], lncf[:], AF.Exp, scale=-1.0)
            meanl = spool.tile([128, 1], F32)
            nc.vector.tensor_tensor(meanl[:], s1b, rcp1[:], OP.mult)

            # unbiased variance, one-pass: (s2 - s1*mean) / max(cnt-1, 1)
            smean = spool.tile([128, 1], F32)
            nc.vector.tensor_tensor(smean[:], s1b, meanl[:], OP.mult)
            diff = spool.tile([128, 1], F32)
            nc.vector.tensor_tensor(diff[:], s2b, smean[:], OP.subtract)
            diffc = spool.tile([128, 1], F32)
            nc.vector.tensor_scalar(diffc[:], diff[:], 0.0, None, OP.max)

            den = spool.tile([128, 1], F32)
            nc.vector.tensor_scalar(
                den[:], cntb, 1.0, 1.0, OP.subtract, OP.max
            )
            lnden = spool.tile([128, 1], F32)
            nc.scalar.activation(lnden[:], den[:], AF.Ln)
            rcp2 = spool.tile([128, 1], F32)
            nc.scalar.activation(rcp2[:], lnden[:], AF.Exp, scale=-1.0)
            varl = spool.tile([128, 1], F32)
            nc.vector.tensor_tensor(varl[:], diffc[:], rcp2[:], OP.mult)

            # count<=1 -> std2 := 1  (stdL := ln2), via varL += ind*ln2^2
            ind = spool.tile([128, 1], F32)
            nc.vector.tensor_scalar(
                ind[:], cntb, 1.5, ln2sq, OP.is_lt, OP.mult
            )
            varp = spool.tile([128, 1], F32)
            nc.vector.tensor_tensor(varp[:], varl[:], ind[:], OP.add)

            # sc = TLS*ln2/sqrt(varp) = exp(-0.5*ln(varp) + ln(TLS*ln2))
            lnv = spool.tile([128, 1], F32)
            nc.scalar.activation(lnv[:], varp[:], AF.Ln)
            ebias = spool.tile([128, 1], F32)
            nc.vector.memset(ebias[:], exp_bias)
            sc_r = spool.tile([128, 1], F32)
            nc.scalar.activation(
                sc_r[:], lnv[:], AF.Exp, scale=-0.5, bias=ebias[:, 0:1]
            )
            sc = spool.tile([128, 1], F32)
            nc.vector.tensor_scalar(sc[:], sc_r[:], sc_clamp, None, OP.min)
            mb = spool.tile([128, 1], F32)
            nc.vector.tensor_tensor(mb[:], meanl[:], sc[:], OP.mult)
            bi = spool.tile([128, 1], F32)
            nc.vector.tensor_scalar(
                bi[:], mb[:], -1.0, LN2_T * TARGET_LOG_MEAN, OP.mult, OP.add
            )

            # ---------------- phase B: streamed elementwise map
            # loads on SP's ring (after phase-A inputs); O1 stores inline on
            # ACT's ring right after each Exp; O2/O3 stores on SP's ring
            # after the loads (SP is idle by then; each waits its Exp sem).
            # Separate rings matter: HWDGE descriptor generation is ~45ns
            # per descriptor per ring, one ring cannot feed loads+stores.
            tiles = []
            f0 = 0
            for ti, (w, slices, tgts) in enumerate(PLAN):
                tl = bpool.tile([128, w], F32, tag=f"t{ti}")
                nc.sync.dma_start(tl[:, :], xst[:, f0 : f0 + w])
                tiles.append((tl, f0))
                f0 += w
            o1 = 0
            o23 = 0
            sp_stores = []
            for (tl, f0), (w, slices, tgts) in zip(tiles, PLAN):
                s0 = 0
                for sw, tgt in zip(slices, tgts):
                    cur = tl[:, s0 : s0 + sw]
                    nc.scalar.activation(cur, cur, AF.Ln)
                    nc.scalar.activation(
                        cur, cur, AF.Exp, scale=sc[:, 0:1], bias=bi[:, 0:1]
                    )
                    if tgt == "O1":
                        nc.scalar.dma_start(y1t[:, o1 : o1 + sw], cur)
                        o1 += sw
                    else:
                        sp_stores.append((cur, tl, s0, sw, o23))
                        o23 += sw
                    s0 += sw
            for cur, tl, s0, sw, oc in sp_stores:
                nc.sync.dma_start(
                    y2t[:, oc : oc + sw], tl[0:120, s0 : s0 + sw]
                )
                nc.sync.dma_start(
                    y3t[:, oc : oc + sw], tl[120:128, s0 : s0 + sw]
                )

    _legalize_sync_waits(nc)
    nc.finalize()
    return nc


_cache = {}


def _get_program(shard, tail):
    key = (shard, tail)
    if key not in _cache:
        _cache[key] = _build_program(shard, tail)
    return _cache[key]


def _consts():
    k = np.arange(128, dtype=np.float32)
    lt_strict = (k[:, None] < k[None, :]).astype(np.float32)  # [k, m]: k < m
    ones = np.ones((128, 128), np.float32)
    return np.concatenate([lt_strict, ones], axis=1)


def _prep(x):
    """Build (nc, in_maps) for the full input x."""
    n = x.shape[0]
    shard = n // N_CORES
    assert n % (N_CORES * 128) == 0, f"unsupported size {n}"

    # tail window guaranteed to contain the last BUF valid elements
    tail = 16384
    while tail < n and int(np.count_nonzero(x[n - tail :] > 0.0)) < BUF:
        tail *= 2
    tail = min(tail, n)
    # phase-A SBUF tiles scale with the tail; beyond 2^16 elements they
    # would not fit alongside the streaming pool
    if tail > (1 << 16):
        # pathological density: synthesize an equivalent tail on the host
        # holding the last <=BUF valid values (stats are order-independent)
        vals = x[x > 0.0]
        kv = vals[-BUF:] if vals.size > BUF else vals
        tail = 16384
        fake = np.zeros(tail, np.float32)
        if kv.size:
            fake[-kv.size :] = kv
        xt = fake
    else:
        xt = x[n - tail :]

    nc = _get_program(shard, tail)
    consts = _consts()
    in_maps = [
        {
            "xs": x[c * SHARD : (c + 1) * SHARD],
            "xt": xt,
            "lts": consts,
        }
        for c in range(N_CORES)
    ]
    return nc, in_maps


def _assemble(res):
    out = np.empty(N_CORES * SHARD, np.float32)
    for c in range(N_CORES):
        v = out[c * SHARD : (c + 1) * SHARD].reshape(128, COLS)
        r = res.results[c]
        v[:, :O1C] = r["y1"].reshape(128, O1C)
        v[0:120, O1C:] = r["y2"].reshape(120, O23C)
        v[120:128, O1C:] = r["y3"].reshape(8, O23C)
    return out


def kernel(pitch_values):
    x = np.ascontiguousarray(np.asarray(pitch_values, dtype=np.float32))
    nc, in_maps = _prep(x)
    res = run_bass_kernel_spmd(nc, in_maps, core_ids=list(range(N_CORES)))
    return _assemble(res)
